# revision 38
# baseline (speedup 1.0000x reference)
"""Causal self-attention Bass kernel for 8 trn2 NeuronCores.

Problem: B=4, T=2048, D=1024, H=16 causal self-attention (qkv proj + attn + out proj).

Sharding: core c = 2*b + g handles batch b (=c//2) and head-group g (=c%2, 8 heads).

Per core (J-outer schedule):
  - x arrives pre-transposed and pre-cast to bf16 from the host as xT [D, T].
  - Attention runs J-outer: for each 512-wide tq chunk J, all 4 head pairs process
    their causal j blocks. Output chunks therefore complete progressively, letting
    the per-chunk output projection and pairwise ReduceScatter overlap attention.
  - All non-attention PE work (v proj, qk proj chunks, out proj) is split into
    small "fill units" consumed one per attention j-step, so the PE never idles
    while the scalar engine (exp) works. Deadline flushes keep the per-engine
    FIFOs deadlock-free.
  - Scores in transposed layout sT[tk, tq]; softmax denominator via a ones-column
    in the AV matmul (psum row 64). Scores / exp / AV trimmed to the causal region
    on diagonal blocks.
  - Normalization: the two [1,512] denominator rows are transposed into lanes with
    DVE 32x32 stream transposes, inverted with reciprocal_approx_fast, transposed
    back, and broadcast to 64/128 partitions via K=1 outer-product matmuls into
    psum (no DRAM round trip, no gpsimd broadcast).
  - Per-chunk ReduceScatter {2b, 2b+1} in bf16; host reassembles and casts to f32.

Precision: all matmul operands bf16, f32 psum accumulation. b_v is folded into
beta = b_proj (even core only) + w_proj_shard.T @ b_v_shard since softmax rows
sum to 1.
"""

from collections import deque
from contextlib import ExitStack

import ml_dtypes
import numpy as np

import concourse.mybir as mybir
import concourse.tile as tile
from concourse import bacc
from concourse.bass_utils import run_bass_kernel_spmd

B, T, D, H = 4, 2048, 1024, 16
HD = D // H  # 64
NCORES = 8
P = 128
f32 = mybir.dt.float32
f32r = mybir.dt.float32r
bf16 = mybir.dt.bfloat16
EXP = mybir.ActivationFunctionType.Exp

_CACHE = {}
LAST_RESULTS = None
_DEBUG_SINK = None


def _dbg(nc, name, ap):
    if _DEBUG_SINK is not None and name in _DEBUG_SINK:
        nc.sync.dma_start(_DEBUG_SINK[name].ap(), ap)


def _emit(nc, tc, xT_d, wqk_d, wv_d, bqk_d, wproj_d, beta_d, out_d):
    with ExitStack() as ctx:
        # ---------------- constants ----------------
        const = ctx.enter_context(tc.tile_pool(name="const", bufs=1))
        mask_tri = const.tile([P, P], bf16, tag="mask_tri")
        nc.gpsimd.memset(mask_tri[:], 1.0)
        nc.gpsimd.affine_select(
            out=mask_tri[:], in_=mask_tri[:],
            compare_op=mybir.AluOpType.is_ge, fill=0.0,
            base=0, pattern=[[1, P]], channel_multiplier=-1,
        )
        bq_all = const.tile([P, 8], f32, tag="bq_all")
        beta_b = const.tile([P, D], bf16, tag="beta_b")
        ones8 = const.tile([P, 8], bf16, tag="ones8")
        nc.vector.memset(ones8[:], 1.0)
        # selector for the K=2 denominator-broadcast matmul:
        # row 0 -> out partitions 0-63 (head A), row 1 -> 64-127 (head B)
        # selectors for the K=33 denominator-broadcast matmuls: selA picks
        # row 0 (head A denom), selB picks row 32 (head B denom)
        selAb = const.tile([33, 64], bf16, tag="selAb")
        nc.gpsimd.memset(selAb[:], 0.0)
        nc.gpsimd.memset(selAb[0:1, :], 1.0)
        selBb = const.tile([33, 64], bf16, tag="selBb")
        nc.gpsimd.memset(selBb[:], 0.0)
        nc.gpsimd.memset(selBb[32:33, :], 1.0)
        selA = const.tile([33, 64], f32r, tag="selA")
        nc.vector.tensor_copy(selA[:], selAb[:])
        selB = const.tile([33, 64], f32r, tag="selB")
        nc.vector.tensor_copy(selB[:], selBb[:])
        # persistent denominator scratch: rows 1-31 and 33-63 stay zero forever
        # so the K=33 broadcast matmuls see clean zeros off the two data rows
        dAB = const.tile([64, 512], f32, tag="dAB")
        nc.vector.memset(dAB[:], 0.0)
        dT = const.tile([64, 512], f32, tag="dT")
        dABr = const.tile([33, 512], f32r, tag="dABr")
        # prewarm the exp table set so the ~2.7us ACT_TABLE_LOAD overlaps the
        # x DMA instead of the first score block
        warm = const.tile([1, 8], bf16, tag="warm")
        nc.scalar.activation(warm[:], ones8[0:1, :], EXP, bias=0.0, scale=0.0)

        wpp = ctx.enter_context(tc.tile_pool(name="wpp", bufs=1))
        wproj_t = [wpp.tile([P, D], bf16, tag=f"wp{hp}", name=f"wp{hp}") for hp in range(4)]

        vv_pool = ctx.enter_context(tc.tile_pool(name="vv", bufs=1))
        vv = [vv_pool.tile([P, 520], bf16, tag=f"vv{i}", name=f"vv{i}") for i in range(16)]
        on_pool = ctx.enter_context(tc.tile_pool(name="outn", bufs=1))
        outN = [[on_pool.tile([P, 512], bf16, tag=f"outN{mp}J{J}", name=f"outN{mp}J{J}")
                 for J in range(4)] for mp in range(4)]
        ones_src = ones8[:].rearrange("p (mp h one) -> p mp h one", mp=4, h=2)
        for i in range(16):
            dst = vv[i][:].rearrange("p (mp h d) -> p mp h d", mp=4, h=2)
            nc.vector.tensor_copy(dst[:, :, :, 64:65], ones_src[:, :, :, :])

        dram = ctx.enter_context(tc.tile_pool(name="dram", bufs=1, space="DRAM"))
        rs_in = [dram.tile([512, D], bf16, tag=f"rsin{Jc}", name=f"rsin{Jc}")
                 for Jc in range(4)]
        rs_out = [dram.tile([256, D], bf16, tag=f"rsout{Jc}", name=f"rsout{Jc}")
                  for Jc in range(4)]

        qkt_pool = ctx.enter_context(tc.tile_pool(name="qkt", bufs=1))
        qkT = [qkt_pool.tile([P, T], bf16, tag=f"qkT{m}", name=f"qkT{m}") for m in range(8)]
        xt_pool = ctx.enter_context(tc.tile_pool(name="xt", bufs=1))
        xT = [xt_pool.tile([P, T], bf16, tag=f"xT{k}", name=f"xT{k}") for k in range(8)]
        wvp = ctx.enter_context(tc.tile_pool(name="wv", bufs=1))
        wv_t = [wvp.tile([P, 512], bf16, tag=f"wvt{k}", name=f"wvt{k}") for k in range(8)]
        wqkp = ctx.enter_context(tc.tile_pool(name="wqk", bufs=1))
        wq_t = [wqkp.tile([P, 1024], bf16, tag=f"wqkt{k}", name=f"wqkt{k}")
                for k in range(8)]
        wps = ctx.enter_context(tc.tile_pool(name="wps", bufs=2, space="PSUM"))

        # ---------------- critical loads first ----------------
        # w_qk columns are host-permuted to m-order (0,4,1,5,2,6,3,7) so the
        # lead-in (m=0,4) needs only the first 256 columns of each k tile.
        # Non-critical loads are emitted after the lead-in chains below.
        for k in range(8):
            nc.sync.dma_start(xT[k][:, 0:1024], xT_d.ap()[k * P : (k + 1) * P, 0:1024])
        for k in range(8):
            nc.scalar.dma_start(wv_t[k][:], wv_d.ap()[k * P : (k + 1) * P, :])
        for k in range(8):
            nc.scalar.dma_start(
                wq_t[k][:, 0:256], wqk_d.ap()[k * P : (k + 1) * P, 0:256]
            )
        nc.scalar.dma_start(bq_all[:], bqk_d.ap())
        MCOL = {0: 0, 4: 1, 1: 2, 5: 3, 2: 4, 6: 5, 3: 6, 7: 7}

        def _load_rest():
            for k in range(8):
                nc.sync.dma_start(
                    xT[k][:, 1024:2048], xT_d.ap()[k * P : (k + 1) * P, 1024:2048]
                )
            for k in range(8):
                nc.scalar.dma_start(
                    wq_t[k][:, 256:1024], wqk_d.ap()[k * P : (k + 1) * P, 256:1024]
                )
            for hp in range(4):
                nc.scalar.dma_start(
                    wproj_t[hp][:], wproj_d.ap()[hp * P : (hp + 1) * P, :]
                )
            nc.scalar.dma_start(beta_b[0:1, :], beta_d.ap())
            nc.gpsimd.partition_broadcast(beta_b[:], beta_b[0:1, :], channels=P)

        # ---------------- work units ----------------
        def v_chain(i, half):
            """half 0/1: 4 of the 8 k-matmuls for v t-tile i; evict on half 1."""
            if half == 0:
                _vbox[i] = wps.tile([P, 512], f32, tag="wp_ps", name=f"vps{i}")
            ps = _vbox[i]
            for k in range(4 * half, 4 * half + 4):
                nc.tensor.matmul(
                    ps[:], xT[k][:, i * P : (i + 1) * P], wv_t[k][:],
                    start=(k == 0), stop=(k == 7),
                )
            if half == 1:
                src = ps[:].rearrange("p (mp h d) -> p mp h d", mp=4, h=2)
                dst = vv[i][:].rearrange("p (mp h d) -> p mp h d", mp=4, h=2)
                nc.vector.tensor_copy(dst[:, :, :, 0:64], src[:, :, :, :])
        _vbox = {}

        _qkbox = {}
        def qk_chain(m, n, quarter):
            """quarter 0..3: 2 of the 8 k-matmuls for qkT[m] chunk n; evict last."""
            if quarter == 0:
                _qkbox[(m, n)] = wps.tile([P, 512], f32, tag="wp_ps", name=f"qps{m}_{n}")
            ps = _qkbox[(m, n)]
            mc = MCOL[m]
            for k in (2 * quarter, 2 * quarter + 1):
                nc.tensor.matmul(
                    ps[:], wq_t[k][:, mc * P : (mc + 1) * P],
                    xT[k][:, n * 512 : (n + 1) * 512],
                    start=(k == 0), stop=(k == 7),
                )
            if quarter == 3:
                nc.vector.tensor_scalar_add(
                    qkT[m][:, n * 512 : (n + 1) * 512], ps[:], bq_all[:, m : m + 1]
                )

        atp = ctx.enter_context(tc.tile_pool(name="atp", bufs=3))

        _pbox = {}
        def proj_unit(Jc, il, n, half):
            """half 0/1: 2 of the 4 hp-matmuls; evict + rs_in write on half 1."""
            if half == 0:
                _pbox[(Jc, il, n)] = wps.tile(
                    [P, 512], f32, tag="wp_ps", name=f"pps{Jc}_{il}_{n}"
                )
            ps = _pbox[(Jc, il, n)]
            for hp in (2 * half, 2 * half + 1):
                nc.tensor.matmul(
                    ps[:],
                    outN[hp][Jc][:, il * P : (il + 1) * P],
                    wproj_t[hp][:, n * 512 : (n + 1) * 512],
                    start=(hp == 0), stop=(hp == 3),
                )
            if half == 1:
                fin = atp.tile([P, 1024], bf16, tag="at", name="fin")
                nc.vector.tensor_add(fin[:, 0:512], ps[:], beta_b[:, n * 512 : (n + 1) * 512])
                nc.sync.dma_start(
                    rs_in[Jc][il * P : (il + 1) * P, n * 512 : (n + 1) * 512],
                    fin[:, 0:512],
                )

        def rs_unit(Jc, sub=None):
            """sub=None: whole 512-row chunk. sub=0/1: 256-row half of the
            chunk (pipelines the serial tail on chunk 3)."""
            if sub is None:
                in_ap = rs_in[Jc].opt()
                orows = slice(0, 256)
            else:
                in_ap = rs_in[Jc][sub * 256 : (sub + 1) * 256, :]
                orows = slice(sub * 128, (sub + 1) * 128)
            if globals().get("_NO_COLLECTIVE"):
                nc.sync.dma_start(
                    rs_out[Jc][orows, :],
                    rs_in[Jc][orows.start * 2 : orows.start * 2 + (orows.stop - orows.start), :],
                )
            else:
                nc.gpsimd.collective_compute(
                    "ReduceScatter", mybir.AluOpType.add,
                    replica_groups=[[0, 1], [2, 3], [4, 5], [6, 7]],
                    ins=[in_ap],
                    outs=[rs_out[Jc][orows, :]],
                )
            # gpsimd queue carries only collectives, so blocking on the RS
            # completion here cannot stall compute
            nc.gpsimd.dma_start(
                out_d.ap()[Jc * 256 + orows.start : Jc * 256 + orows.stop, :],
                rs_out[Jc][orows, :],
            )

        # fill queue: (tag, thunk) consumed one per attention j-step
        pending = deque()

        def pump():
            if pending:
                tag, thunk = pending.popleft()
                thunk()

        def flush(pred):
            """Emit from the front until no pending unit matches pred."""
            while any(pred(tag) for tag, _ in pending):
                tag, thunk = pending.popleft()
                thunk()

        def flush_all():
            while pending:
                tag, thunk = pending.popleft()
                thunk()

        # ---------------- lead-in: v tiles 0-3 and qk chunk-0 chains ----------
        for i in range(4):
            v_chain(i, 0); v_chain(i, 1)
        for m in (0, 4):
            for qq in range(4):
                qk_chain(m, 0, qq)
        _load_rest()
        for m in (1, 5, 2, 6):
            for qq in range(4):
                qk_chain(m, 0, qq)
        for m in (3, 7):
            for qq in range(4):
                pending.append((("qk", m, 0), (lambda m=m, qq=qq: qk_chain(m, 0, qq))))
        for i in range(4, 8):
            for half in range(2):
                pending.append((("v", i), (lambda i=i, h=half: v_chain(i, h))))
        for m in (0, 4, 1, 5, 2, 6, 3, 7):
            for qq in range(4):
                pending.append((("qk", m, 1), (lambda m=m, qq=qq: qk_chain(m, 1, qq))))

        # ---------------- attention: J-outer over tq chunks ----------------
        recip = ctx.enter_context(tc.tile_pool(name="recip", bufs=1))
        tmpb = ctx.enter_context(tc.tile_pool(name="tmpb", bufs=1))
        stps = ctx.enter_context(tc.tile_pool(name="stps", bufs=2, space="PSUM"))
        oups = ctx.enter_context(tc.tile_pool(name="oups", bufs=1, space="PSUM"))

        for J in range(4):
            nj = 4 * J + 4
            # correctness: everything this J's emission depends on must be
            # emitted first (per-engine FIFOs would deadlock otherwise)
            flush(lambda tag: tag[0] == "v" and tag[1] <= 4 * J + 3)
            for mp in range(4):
                flush(lambda tag: tag[0] == "qk" and tag[2] == J
                      and tag[1] in (mp, 4 + mp))
                qs, ks = qkT[mp], qkT[4 + mp]
                ouA = oups.tile([65, 512], f32, tag="ouA")
                ouB = oups.tile([65, 512], f32, tag="ouB")
                for j in range(nj):
                    sT = stps.tile([P, 1024], f32, tag="sT")
                    js = slice(j * P, (j + 1) * P)
                    i = j - 4 * J
                    c0 = 128 * i if i > 0 else 0
                    qcols = slice(J * 512 + c0, (J + 1) * 512)
                    nc.tensor.matmul(
                        sT[:, c0:512],
                        ks[0:64, js], qs[0:64, qcols],
                        start=True, stop=True, tile_position=(0, 0),
                    )
                    nc.tensor.matmul(
                        sT[:, 512 + c0 : 1024],
                        ks[64:128, js], qs[64:128, qcols],
                        start=True, stop=True, tile_position=(64, 0),
                    )
                    at = atp.tile([P, 1024], bf16, tag="at")
                    if i > 0:
                        src_v = sT[:].rearrange("p (h c) -> p h c", h=2)
                        dst_v = at[:].rearrange("p (h c) -> p h c", h=2)
                        nc.scalar.activation(
                            dst_v[:, :, c0:512], src_v[:, :, c0:512],
                            EXP, bias=0.0, scale=0.125,
                        )
                    else:
                        nc.scalar.activation(at[:], sT[:], EXP, bias=0.0, scale=0.125)
                    if i >= 0:
                        for h0 in (0, 512):
                            nc.vector.tensor_mul(
                                at[:, h0 + c0 : h0 + c0 + 128],
                                at[:, h0 + c0 : h0 + c0 + 128], mask_tri[:],
                            )
                    if mp == 0 and J == 0 and j == 0:
                        _dbg(nc, "at000", at[:])
                    nc.tensor.matmul(
                        ouA[:, c0:512], vv[j][:, 130 * mp : 130 * mp + 65],
                        at[:, c0:512],
                        start=(j == 0), stop=(j == nj - 1),
                        skip_group_check=True,
                    )
                    nc.tensor.matmul(
                        ouB[:, c0:512], vv[j][:, 130 * mp + 65 : 130 * mp + 130],
                        at[:, 512 + c0 : 1024],
                        start=(j == 0), stop=(j == nj - 1),
                        skip_group_check=True,
                    )
                    pump()
                # ---- normalize (mp, J): raw evict, lane-transposed
                # reciprocal, matmul broadcast, scale ----
                tb = tmpb.tile([64, 512], bf16, tag="tb")
                nc.vector.tensor_copy(dAB[0:1, :], ouA[64:65, :])
                nc.vector.tensor_copy(outN[mp][J][0:64, :], ouA[0:64, :])
                nc.vector.tensor_copy(dAB[32:33, :], ouB[64:65, :])
                nc.vector.tensor_copy(tb[:], ouB[0:64, :])
                # lane-transpose the two denominator rows (-> col 0 of each
                # 32x32 block), invert on 64 lanes, transpose back
                nc.vector.transpose(dT[:], dAB[:])
                sel = dT[:].rearrange("p (b c) -> p b c", b=16)[:, :, 0:1]
                nc.vector.reciprocal_approx_fast(sel, sel)
                nc.vector.transpose(dAB[:], dT[:])
                nc.vector.tensor_copy(dABr[:], dAB[0:33, :])
                bcpA = wps.tile([P, 512], f32, tag="wp_ps", name="bcpA")
                nc.tensor.matmul(
                    bcpA[0:64, :], selA[:], dABr[:], start=True, stop=True,
                )
                bcpB = wps.tile([P, 512], f32, tag="wp_ps", name="bcpB")
                nc.tensor.matmul(
                    bcpB[0:64, :], selB[:], dABr[:], start=True, stop=True,
                )
                nc.vector.tensor_mul(outN[mp][J][0:64, :], outN[mp][J][0:64, :], bcpA[0:64, :])
                nc.vector.tensor_mul(tb[:], tb[:], bcpB[0:64, :])
                # DMA-shift head B rows up; last step, so the DVE chain never
                # waits on the sync queue
                nc.sync.dma_start(outN[mp][J][64:128, :], tb[:])
            # ---- chunk J complete: queue its projection + ReduceScatter,
            # v tiles for chunk J+2, and the qk chains needed by chunk J+2 ----
            if J == 0:
                for i in range(8, 12):
                    for half in range(2):
                        pending.append((("v", i), (lambda i=i, h=half: v_chain(i, h))))
            if J == 1:
                for i in range(12, 16):
                    for half in range(2):
                        pending.append((("v", i), (lambda i=i, h=half: v_chain(i, h))))
            if J < 2:
                for m in (0, 4, 1, 5, 2, 6, 3, 7):
                    for qq in range(4):
                        pending.append((("qk", m, J + 2), (lambda m=m, qq=qq, n=J + 2: qk_chain(m, n, qq))))
            for il in range(4):
                for n in range(2):
                    for half in range(2):
                        pending.append((("proj", J), (lambda Jc=J, il=il, n=n, h=half: proj_unit(Jc, il, n, h))))
                if J == 3 and il == 1:
                    pending.append((("rs", J), (lambda: rs_unit(3, 0))))
            if J == 3:
                pending.append((("rs", J), (lambda: rs_unit(3, 1))))
            else:
                pending.append((("rs", J), (lambda Jc=J: rs_unit(Jc))))

        flush_all()
        _dbg(nc, "qkT0", qkT[0][:])
        _dbg(nc, "qkT4", qkT[4][:])
        _dbg(nc, "xT0", xT[0][:])
        _dbg(nc, "vv0", vv[0][:])
        if _DEBUG_SINK is not None and "outN0" in _DEBUG_SINK:
            for J in range(4):
                nc.sync.dma_start(
                    _DEBUG_SINK["outN0"].ap()[:, J * 512 : (J + 1) * 512],
                    outN[0][J][:],
                )
        for Jc in range(4):
            _dbg(nc, f"rs_in{Jc}", rs_in[Jc][:])


def _build():
    if "nc" in _CACHE:
        return _CACHE["nc"]
    nc = bacc.Bacc("TRN2", target_bir_lowering=False, debug=False, num_devices=NCORES)
    xT_d = nc.dram_tensor("xT", [D, T], bf16, kind="ExternalInput")
    wqk_d = nc.dram_tensor("w_qk", [D, 1024], bf16, kind="ExternalInput")
    wv_d = nc.dram_tensor("w_v", [D, 512], bf16, kind="ExternalInput")
    bqk_d = nc.dram_tensor("b_qk", [P, 8], f32, kind="ExternalInput")
    wproj_d = nc.dram_tensor("w_proj", [512, D], bf16, kind="ExternalInput")
    beta_d = nc.dram_tensor("beta", [1, D], bf16, kind="ExternalInput")
    out_d = nc.dram_tensor("out", [T // 2, D], bf16, kind="ExternalOutput")
    with tile.TileContext(nc) as tc:
        _emit(nc, tc, xT_d, wqk_d, wv_d, bqk_d, wproj_d, beta_d, out_d)
    nc.compile()
    _CACHE["nc"] = nc
    return nc


def make_in_maps(x, w_qkv, b_qkv, w_proj, b_proj):
    x = np.asarray(x, np.float32)
    w_qkv = np.asarray(w_qkv, np.float32)
    b_qkv = np.asarray(b_qkv, np.float32)
    w_proj = np.asarray(w_proj, np.float32)
    b_proj = np.asarray(b_proj, np.float32)
    in_maps = []
    for c in range(NCORES):
        b, g = c // 2, c % 2
        qcols = slice(g * 512, (g + 1) * 512)
        kcols = slice(D + g * 512, D + (g + 1) * 512)
        vcols = slice(2 * D + g * 512, 2 * D + (g + 1) * 512)
        w_qk = np.concatenate([w_qkv[:, qcols], w_qkv[:, kcols]], axis=1)
        # permute 128-col blocks to m-order (0,4,1,5,2,6,3,7): the kernel's
        # lead-in then only needs the first 256 columns per k tile
        blk = w_qk.reshape(D, 8, P)
        w_qk = blk[:, [0, 4, 1, 5, 2, 6, 3, 7], :].reshape(D, 1024)
        b_qk = np.concatenate([b_qkv[qcols], b_qkv[kcols]])
        wp = np.ascontiguousarray(w_proj[g * 512 : (g + 1) * 512, :])
        beta = wp.T @ b_qkv[vcols]
        if g == 0:
            beta = beta + b_proj
        in_maps.append({
            "xT": np.ascontiguousarray(x[b].T).astype(ml_dtypes.bfloat16),
            "w_qk": np.ascontiguousarray(w_qk).astype(ml_dtypes.bfloat16),
            "w_v": np.ascontiguousarray(w_qkv[:, vcols]).astype(ml_dtypes.bfloat16),
            "b_qk": np.ascontiguousarray(b_qk.reshape(8, P).T),
            "w_proj": wp.astype(ml_dtypes.bfloat16),
            "beta": beta.reshape(1, D).astype(ml_dtypes.bfloat16),
        })
    return in_maps


def kernel(x, w_qkv, b_qkv, w_proj, b_proj, trace=False, **run_kwargs):
    global LAST_RESULTS
    nc = _build()
    in_maps = make_in_maps(x, w_qkv, b_qkv, w_proj, b_proj)
    res = run_bass_kernel_spmd(
        nc, in_maps, core_ids=list(range(NCORES)), trace=trace, **run_kwargs
    )
    LAST_RESULTS = res
    out = np.empty((B, T, D), np.float32)
    for b in range(B):
        e = res.results[2 * b]["out"].astype(np.float32)
        o = res.results[2 * b + 1]["out"].astype(np.float32)
        for Jc in range(3):
            out[b, 512 * Jc : 512 * Jc + 256] = e[256 * Jc : 256 * (Jc + 1)]
            out[b, 512 * Jc + 256 : 512 * (Jc + 1)] = o[256 * Jc : 256 * (Jc + 1)]
        # chunk 3 ran as two 256-row half-chunks
        out[b, 1536:1664] = e[768:896]
        out[b, 1664:1792] = o[768:896]
        out[b, 1792:1920] = e[896:1024]
        out[b, 1920:2048] = o[896:1024]
    return out


# revision 42
# speedup vs baseline: 1.0765x; 1.0765x over previous
"""Causal self-attention Bass kernel for 8 trn2 NeuronCores.

Problem: B=4, T=2048, D=1024, H=16 causal self-attention (qkv proj + attn + out proj).

Sharding: core c = 2*b + g handles batch b (=c//2) and head-group g (=c%2, 8 heads).

Per core (J-outer schedule):
  - x arrives pre-transposed and pre-cast to bf16 from the host as xT [D, T].
  - Attention runs J-outer: for each 512-wide tq chunk J, all 4 head pairs process
    their causal j blocks. Output chunks therefore complete progressively, letting
    the per-chunk output projection and pairwise ReduceScatter overlap attention.
  - All non-attention PE work (v proj, qk proj chunks, out proj) is split into
    small "fill units" consumed one per attention j-step, so the PE never idles
    while the scalar engine (exp) works. Deadline flushes keep the per-engine
    FIFOs deadlock-free.
  - Scores in transposed layout sT[tk, tq]; softmax denominator via a ones-column
    in the AV matmul (psum row 64). Scores / exp / AV trimmed to the causal region
    on diagonal blocks.
  - Normalization: the two [1,512] denominator rows are transposed into lanes with
    DVE 32x32 stream transposes, inverted with reciprocal_approx_fast, transposed
    back, and broadcast to 64/128 partitions via K=1 outer-product matmuls into
    psum (no DRAM round trip, no gpsimd broadcast).
  - Per-chunk ReduceScatter {2b, 2b+1} in bf16; host reassembles and casts to f32.

Precision: all matmul operands bf16, f32 psum accumulation. b_v is folded into
beta = b_proj (even core only) + w_proj_shard.T @ b_v_shard since softmax rows
sum to 1.
"""

from collections import deque
from contextlib import ExitStack

import ml_dtypes
import numpy as np

import concourse.mybir as mybir
import concourse.tile as tile
from concourse import bacc
from concourse.bass_utils import run_bass_kernel_spmd

B, T, D, H = 4, 2048, 1024, 16
HD = D // H  # 64
NCORES = 8
P = 128
f32 = mybir.dt.float32
f32r = mybir.dt.float32r
bf16 = mybir.dt.bfloat16
EXP = mybir.ActivationFunctionType.Exp

_CACHE = {}
LAST_RESULTS = None
_DEBUG_SINK = None


def _dbg(nc, name, ap):
    if _DEBUG_SINK is not None and name in _DEBUG_SINK:
        nc.sync.dma_start(_DEBUG_SINK[name].ap(), ap)


def _emit(nc, tc, xT_d, wqk_d, wv_d, bqk_d, wproj_d, beta_d, out_d):
    with ExitStack() as ctx:
        # ---------------- constants ----------------
        const = ctx.enter_context(tc.tile_pool(name="const", bufs=1))
        mask_tri = const.tile([P, P], bf16, tag="mask_tri")
        nc.gpsimd.memset(mask_tri[:], 1.0)
        nc.gpsimd.affine_select(
            out=mask_tri[:], in_=mask_tri[:],
            compare_op=mybir.AluOpType.is_ge, fill=0.0,
            base=0, pattern=[[1, P]], channel_multiplier=-1,
        )
        bq_all = const.tile([P, 8], f32, tag="bq_all")
        beta_b = const.tile([P, D], bf16, tag="beta_b")
        ones8 = const.tile([P, 8], bf16, tag="ones8")
        nc.vector.memset(ones8[:], 1.0)
        # selector for the K=2 denominator-broadcast matmul:
        # row 0 -> out partitions 0-63 (head A), row 1 -> 64-127 (head B)
        # selectors for the K=33 denominator-broadcast matmuls: selA picks
        # row 0 (head A denom), selB picks row 32 (head B denom)
        selAb = const.tile([33, 64], bf16, tag="selAb")
        nc.gpsimd.memset(selAb[:], 0.0)
        nc.gpsimd.memset(selAb[0:1, :], 1.0)
        selBb = const.tile([33, 64], bf16, tag="selBb")
        nc.gpsimd.memset(selBb[:], 0.0)
        nc.gpsimd.memset(selBb[32:33, :], 1.0)
        selA = const.tile([33, 64], f32r, tag="selA")
        nc.vector.tensor_copy(selA[:], selAb[:])
        selB = const.tile([33, 64], f32r, tag="selB")
        nc.vector.tensor_copy(selB[:], selBb[:])
        # persistent denominator scratch: rows 1-31 and 33-63 stay zero forever
        # so the K=33 broadcast matmuls see clean zeros off the two data rows
        dAB = const.tile([64, 512], f32, tag="dAB")
        nc.vector.memset(dAB[:], 0.0)
        dT = const.tile([64, 512], f32, tag="dT")
        dABr = const.tile([33, 512], f32r, tag="dABr")
        # prewarm the exp table set so the ~2.7us ACT_TABLE_LOAD overlaps the
        # x DMA instead of the first score block
        warm = const.tile([1, 8], bf16, tag="warm")
        nc.scalar.activation(warm[:], ones8[0:1, :], EXP, bias=0.0, scale=0.0)

        wpp = ctx.enter_context(tc.tile_pool(name="wpp", bufs=1))
        wproj_t = [wpp.tile([P, D], bf16, tag=f"wp{hp}", name=f"wp{hp}") for hp in range(4)]

        vv_pool = ctx.enter_context(tc.tile_pool(name="vv", bufs=1))
        vv = [vv_pool.tile([P, 520], bf16, tag=f"vv{i}", name=f"vv{i}") for i in range(16)]
        on_pool = ctx.enter_context(tc.tile_pool(name="outn", bufs=1))
        outN = [[on_pool.tile([P, 512], bf16, tag=f"outN{mp}J{J}", name=f"outN{mp}J{J}")
                 for J in range(4)] for mp in range(4)]
        ones_src = ones8[:].rearrange("p (mp h one) -> p mp h one", mp=4, h=2)
        for i in range(16):
            dst = vv[i][:].rearrange("p (mp h d) -> p mp h d", mp=4, h=2)
            nc.vector.tensor_copy(dst[:, :, :, 64:65], ones_src[:, :, :, :])

        dram = ctx.enter_context(tc.tile_pool(name="dram", bufs=1, space="DRAM"))
        rs_in = [dram.tile([512, D], bf16, tag=f"rsin{Jc}", name=f"rsin{Jc}")
                 for Jc in range(4)]
        rs_out = [dram.tile([256, D], bf16, tag=f"rsout{Jc}", name=f"rsout{Jc}")
                  for Jc in range(4)]

        qkt_pool = ctx.enter_context(tc.tile_pool(name="qkt", bufs=1))
        qkT = [qkt_pool.tile([P, T], bf16, tag=f"qkT{m}", name=f"qkT{m}") for m in range(8)]
        xt_pool = ctx.enter_context(tc.tile_pool(name="xt", bufs=1))
        xT = [xt_pool.tile([P, T], bf16, tag=f"xT{k}", name=f"xT{k}") for k in range(8)]
        wvp = ctx.enter_context(tc.tile_pool(name="wv", bufs=1))
        wv_t = [wvp.tile([P, 512], bf16, tag=f"wvt{k}", name=f"wvt{k}") for k in range(8)]
        wqkp = ctx.enter_context(tc.tile_pool(name="wqk", bufs=1))
        wq_t = [wqkp.tile([P, 1024], bf16, tag=f"wqkt{k}", name=f"wqkt{k}")
                for k in range(8)]
        wps = ctx.enter_context(tc.tile_pool(name="wps", bufs=2, space="PSUM"))

        # ---------------- critical loads first ----------------
        # w_qk columns are host-permuted to m-order (0,4,1,5,2,6,3,7) so the
        # lead-in (m=0,4) needs only the first 256 columns of each k tile.
        # Non-critical loads are emitted after the lead-in chains below.
        for k in range(8):
            nc.sync.dma_start(xT[k][:, 0:1024], xT_d.ap()[k * P : (k + 1) * P, 0:1024])
        for k in range(8):
            nc.scalar.dma_start(wv_t[k][:], wv_d.ap()[k * P : (k + 1) * P, :])
        for k in range(8):
            nc.scalar.dma_start(
                wq_t[k][:, 0:256], wqk_d.ap()[k * P : (k + 1) * P, 0:256]
            )
        nc.scalar.dma_start(bq_all[:], bqk_d.ap())
        MCOL = {0: 0, 4: 1, 1: 2, 5: 3, 2: 4, 6: 5, 3: 6, 7: 7}

        def _load_rest():
            for k in range(8):
                nc.sync.dma_start(
                    xT[k][:, 1024:2048], xT_d.ap()[k * P : (k + 1) * P, 1024:2048]
                )
            for k in range(8):
                nc.scalar.dma_start(
                    wq_t[k][:, 256:1024], wqk_d.ap()[k * P : (k + 1) * P, 256:1024]
                )
            for hp in range(4):
                nc.scalar.dma_start(
                    wproj_t[hp][:], wproj_d.ap()[hp * P : (hp + 1) * P, :]
                )
            nc.scalar.dma_start(beta_b[0:1, :], beta_d.ap())
            nc.gpsimd.partition_broadcast(beta_b[:], beta_b[0:1, :], channels=P)

        # ---------------- work units ----------------
        def v_chain(i, half):
            """half 0/1: 4 of the 8 k-matmuls for v t-tile i; evict on half 1."""
            if half == 0:
                _vbox[i] = wps.tile([P, 512], f32, tag="wp_ps", name=f"vps{i}")
            ps = _vbox[i]
            for k in range(4 * half, 4 * half + 4):
                nc.tensor.matmul(
                    ps[:], xT[k][:, i * P : (i + 1) * P], wv_t[k][:],
                    start=(k == 0), stop=(k == 7),
                )
            if half == 1:
                src = ps[:].rearrange("p (mp h d) -> p mp h d", mp=4, h=2)
                dst = vv[i][:].rearrange("p (mp h d) -> p mp h d", mp=4, h=2)
                nc.vector.tensor_copy(dst[:, :, :, 0:64], src[:, :, :, :])
        _vbox = {}

        _qkbox = {}
        def qk_chain(m, n, quarter):
            """quarter 0..3: 2 of the 8 k-matmuls for qkT[m] chunk n; evict last."""
            if quarter == 0:
                _qkbox[(m, n)] = wps.tile([P, 512], f32, tag="wp_ps", name=f"qps{m}_{n}")
            ps = _qkbox[(m, n)]
            mc = MCOL[m]
            for k in (2 * quarter, 2 * quarter + 1):
                nc.tensor.matmul(
                    ps[:], wq_t[k][:, mc * P : (mc + 1) * P],
                    xT[k][:, n * 512 : (n + 1) * 512],
                    start=(k == 0), stop=(k == 7),
                )
            if quarter == 3:
                nc.vector.tensor_scalar_add(
                    qkT[m][:, n * 512 : (n + 1) * 512], ps[:], bq_all[:, m : m + 1]
                )

        atp = ctx.enter_context(tc.tile_pool(name="atp", bufs=3))

        _pbox = {}
        def proj_unit(Jc, il, n, half):
            """half 0/1: 2 of the 4 hp-matmuls; evict + rs_in write on half 1."""
            if half == 0:
                _pbox[(Jc, il, n)] = wps.tile(
                    [P, 512], f32, tag="wp_ps", name=f"pps{Jc}_{il}_{n}"
                )
            ps = _pbox[(Jc, il, n)]
            for hp in (2 * half, 2 * half + 1):
                nc.tensor.matmul(
                    ps[:],
                    outN[hp][Jc][:, il * P : (il + 1) * P],
                    wproj_t[hp][:, n * 512 : (n + 1) * 512],
                    start=(hp == 0), stop=(hp == 3),
                )
            if half == 1:
                fin = atp.tile([P, 1024], bf16, tag="at", name="fin")
                nc.vector.tensor_add(fin[:, 0:512], ps[:], beta_b[:, n * 512 : (n + 1) * 512])
                nc.sync.dma_start(
                    rs_in[Jc][il * P : (il + 1) * P, n * 512 : (n + 1) * 512],
                    fin[:, 0:512],
                )

        def rs_unit(Jc, sub=None):
            """sub=None: whole 512-row chunk. sub=0/1: 256-row half of the
            chunk (pipelines the serial tail on chunk 3)."""
            if sub is None:
                in_ap = rs_in[Jc].opt()
                orows = slice(0, 256)
            else:
                in_ap = rs_in[Jc][sub * 256 : (sub + 1) * 256, :]
                orows = slice(sub * 128, (sub + 1) * 128)
            if globals().get("_NO_COLLECTIVE"):
                nc.sync.dma_start(
                    rs_out[Jc][orows, :],
                    rs_in[Jc][orows.start * 2 : orows.start * 2 + (orows.stop - orows.start), :],
                )
            else:
                nc.gpsimd.collective_compute(
                    "ReduceScatter", mybir.AluOpType.add,
                    replica_groups=[[0, 1], [2, 3], [4, 5], [6, 7]],
                    ins=[in_ap],
                    outs=[rs_out[Jc][orows, :]],
                )
            # gpsimd queue carries only collectives, so blocking on the RS
            # completion here cannot stall compute
            nc.gpsimd.dma_start(
                out_d.ap()[Jc * 256 + orows.start : Jc * 256 + orows.stop, :],
                rs_out[Jc][orows, :],
            )

        # fill queue: (tag, thunk) consumed one per attention j-step
        pending = deque()

        def pump():
            if pending:
                tag, thunk = pending.popleft()
                thunk()

        def flush(pred):
            """Emit from the front until no pending unit matches pred."""
            while any(pred(tag) for tag, _ in pending):
                tag, thunk = pending.popleft()
                thunk()

        def flush_all():
            while pending:
                tag, thunk = pending.popleft()
                thunk()

        # ---------------- lead-in: v tiles 0-3 and qk chunk-0 chains ----------
        for i in range(4):
            v_chain(i, 0); v_chain(i, 1)
        for m in (0, 4):
            for qq in range(4):
                qk_chain(m, 0, qq)
        _load_rest()
        for m in (1, 5, 2, 6):
            for qq in range(4):
                qk_chain(m, 0, qq)
        for m in (3, 7):
            for qq in range(4):
                pending.append((("qk", m, 0), (lambda m=m, qq=qq: qk_chain(m, 0, qq))))
        for i in range(4, 8):
            for half in range(2):
                pending.append((("v", i), (lambda i=i, h=half: v_chain(i, h))))
        for m in (0, 4, 1, 5, 2, 6, 3, 7):
            for qq in range(4):
                pending.append((("qk", m, 1), (lambda m=m, qq=qq: qk_chain(m, 1, qq))))

        # ---------------- attention: J-outer over tq chunks ----------------
        recip = ctx.enter_context(tc.tile_pool(name="recip", bufs=1))
        tmpb = ctx.enter_context(tc.tile_pool(name="tmpb", bufs=2))
        stps = ctx.enter_context(tc.tile_pool(name="stps", bufs=2, space="PSUM"))
        oups = ctx.enter_context(tc.tile_pool(name="oups", bufs=1, space="PSUM"))

        for J in range(4):
            nj = 4 * J + 4
            # correctness: everything this J's emission depends on must be
            # emitted first (per-engine FIFOs would deadlock otherwise)
            flush(lambda tag: tag[0] == "v" and tag[1] <= 4 * J + 3)
            for mp in range(4):
                flush(lambda tag: tag[0] == "qk" and tag[2] == J
                      and tag[1] in (mp, 4 + mp))
                qs, ks = qkT[mp], qkT[4 + mp]
                ouA = oups.tile([65, 512], f32, tag="ouA")
                ouB = oups.tile([65, 512], f32, tag="ouB")
                for j in range(nj):
                    sT = stps.tile([P, 1024], f32, tag="sT")
                    js = slice(j * P, (j + 1) * P)
                    i = j - 4 * J
                    c0 = 128 * i if i > 0 else 0
                    qcols = slice(J * 512 + c0, (J + 1) * 512)
                    nc.tensor.matmul(
                        sT[:, c0:512],
                        ks[0:64, js], qs[0:64, qcols],
                        start=True, stop=True, tile_position=(0, 0),
                    )
                    nc.tensor.matmul(
                        sT[:, 512 + c0 : 1024],
                        ks[64:128, js], qs[64:128, qcols],
                        start=True, stop=True, tile_position=(64, 0),
                    )
                    at = atp.tile([P, 1024], bf16, tag="at")
                    if i > 0:
                        src_v = sT[:].rearrange("p (h c) -> p h c", h=2)
                        dst_v = at[:].rearrange("p (h c) -> p h c", h=2)
                        nc.scalar.activation(
                            dst_v[:, :, c0:512], src_v[:, :, c0:512],
                            EXP, bias=0.0, scale=0.125,
                        )
                    else:
                        nc.scalar.activation(at[:], sT[:], EXP, bias=0.0, scale=0.125)
                    if i >= 0:
                        for h0 in (0, 512):
                            nc.vector.tensor_mul(
                                at[:, h0 + c0 : h0 + c0 + 128],
                                at[:, h0 + c0 : h0 + c0 + 128], mask_tri[:],
                            )
                    if mp == 0 and J == 0 and j == 0:
                        _dbg(nc, "at000", at[:])
                    nc.tensor.matmul(
                        ouA[:, c0:512], vv[j][:, 130 * mp : 130 * mp + 65],
                        at[:, c0:512],
                        start=(j == 0), stop=(j == nj - 1),
                        skip_group_check=True,
                    )
                    nc.tensor.matmul(
                        ouB[:, c0:512], vv[j][:, 130 * mp + 65 : 130 * mp + 130],
                        at[:, 512 + c0 : 1024],
                        start=(j == 0), stop=(j == nj - 1),
                        skip_group_check=True,
                    )
                    pump()
                # ---- normalize (mp, J): raw evict, lane-transposed
                # reciprocal, matmul broadcast, scale ----
                # inline: free the psum accumulators fast, split across DVE
                # (denominator rows) and ACT (value blocks) for low latency
                tb = tmpb.tile([64, 512], bf16, tag="tb")
                nc.vector.tensor_copy(dAB[0:1, :], ouA[64:65, :])
                nc.vector.tensor_copy(dAB[32:33, :], ouB[64:65, :])
                nc.scalar.copy(outN[mp][J][0:64, :], ouA[0:64, :])
                nc.scalar.copy(tb[:], ouB[0:64, :])

                def norm_tail(mp=mp, J=J, tb=tb):
                    # lane-transpose the two denominator rows (-> col 0 of
                    # each 32x32 block), invert on 64 lanes, transpose back
                    nc.vector.transpose(dT[:], dAB[:])
                    sel = dT[:].rearrange("p (b c) -> p b c", b=16)[:, :, 0:1]
                    nc.vector.reciprocal_approx_fast(sel, sel)
                    nc.vector.transpose(dAB[:], dT[:])
                    nc.vector.tensor_copy(dABr[:], dAB[0:33, :])
                    bcpA = wps.tile([P, 512], f32, tag="wp_ps", name="bcpA")
                    nc.tensor.matmul(
                        bcpA[0:64, :], selA[:], dABr[:], start=True, stop=True,
                    )
                    bcpB = wps.tile([P, 512], f32, tag="wp_ps", name="bcpB")
                    nc.tensor.matmul(
                        bcpB[0:64, :], selB[:], dABr[:], start=True, stop=True,
                    )
                    nc.vector.tensor_mul(outN[mp][J][0:64, :], outN[mp][J][0:64, :], bcpA[0:64, :])
                    nc.vector.tensor_mul(tb[:], tb[:], bcpB[0:64, :])
                    # DMA-shift head B rows up; last step, so the DVE chain
                    # never waits on the sync queue
                    nc.sync.dma_start(outN[mp][J][64:128, :], tb[:])
                # deferred off the mp-boundary critical path: runs a j-step
                # later, so the next head pair's scores aren't queued behind it
                pending.appendleft((("norm", mp, J), norm_tail))
            # ---- chunk J complete: queue its projection + ReduceScatter,
            # v tiles for chunk J+2, and the qk chains needed by chunk J+2 ----
            if J == 0:
                for i in range(8, 12):
                    for half in range(2):
                        pending.append((("v", i), (lambda i=i, h=half: v_chain(i, h))))
            if J == 1:
                for i in range(12, 16):
                    for half in range(2):
                        pending.append((("v", i), (lambda i=i, h=half: v_chain(i, h))))
            if J < 2:
                for m in (0, 4, 1, 5, 2, 6, 3, 7):
                    for qq in range(4):
                        pending.append((("qk", m, J + 2), (lambda m=m, qq=qq, n=J + 2: qk_chain(m, n, qq))))
            for il in range(4):
                for n in range(2):
                    for half in range(2):
                        pending.append((("proj", J), (lambda Jc=J, il=il, n=n, h=half: proj_unit(Jc, il, n, h))))
            pending.append((("rs", J), (lambda Jc=J: rs_unit(Jc))))

        flush_all()
        _dbg(nc, "qkT0", qkT[0][:])
        _dbg(nc, "qkT4", qkT[4][:])
        _dbg(nc, "xT0", xT[0][:])
        _dbg(nc, "vv0", vv[0][:])
        if _DEBUG_SINK is not None and "outN0" in _DEBUG_SINK:
            for J in range(4):
                nc.sync.dma_start(
                    _DEBUG_SINK["outN0"].ap()[:, J * 512 : (J + 1) * 512],
                    outN[0][J][:],
                )
        for Jc in range(4):
            _dbg(nc, f"rs_in{Jc}", rs_in[Jc][:])


def _build():
    if "nc" in _CACHE:
        return _CACHE["nc"]
    nc = bacc.Bacc("TRN2", target_bir_lowering=False, debug=False, num_devices=NCORES)
    xT_d = nc.dram_tensor("xT", [D, T], bf16, kind="ExternalInput")
    wqk_d = nc.dram_tensor("w_qk", [D, 1024], bf16, kind="ExternalInput")
    wv_d = nc.dram_tensor("w_v", [D, 512], bf16, kind="ExternalInput")
    bqk_d = nc.dram_tensor("b_qk", [P, 8], f32, kind="ExternalInput")
    wproj_d = nc.dram_tensor("w_proj", [512, D], bf16, kind="ExternalInput")
    beta_d = nc.dram_tensor("beta", [1, D], bf16, kind="ExternalInput")
    out_d = nc.dram_tensor("out", [T // 2, D], bf16, kind="ExternalOutput")
    with tile.TileContext(nc) as tc:
        _emit(nc, tc, xT_d, wqk_d, wv_d, bqk_d, wproj_d, beta_d, out_d)
    nc.compile()
    _CACHE["nc"] = nc
    return nc


def make_in_maps(x, w_qkv, b_qkv, w_proj, b_proj):
    x = np.asarray(x, np.float32)
    w_qkv = np.asarray(w_qkv, np.float32)
    b_qkv = np.asarray(b_qkv, np.float32)
    w_proj = np.asarray(w_proj, np.float32)
    b_proj = np.asarray(b_proj, np.float32)
    in_maps = []
    for c in range(NCORES):
        b, g = c // 2, c % 2
        qcols = slice(g * 512, (g + 1) * 512)
        kcols = slice(D + g * 512, D + (g + 1) * 512)
        vcols = slice(2 * D + g * 512, 2 * D + (g + 1) * 512)
        w_qk = np.concatenate([w_qkv[:, qcols], w_qkv[:, kcols]], axis=1)
        # permute 128-col blocks to m-order (0,4,1,5,2,6,3,7): the kernel's
        # lead-in then only needs the first 256 columns per k tile
        blk = w_qk.reshape(D, 8, P)
        w_qk = blk[:, [0, 4, 1, 5, 2, 6, 3, 7], :].reshape(D, 1024)
        b_qk = np.concatenate([b_qkv[qcols], b_qkv[kcols]])
        wp = np.ascontiguousarray(w_proj[g * 512 : (g + 1) * 512, :])
        beta = wp.T @ b_qkv[vcols]
        if g == 0:
            beta = beta + b_proj
        in_maps.append({
            "xT": np.ascontiguousarray(x[b].T).astype(ml_dtypes.bfloat16),
            "w_qk": np.ascontiguousarray(w_qk).astype(ml_dtypes.bfloat16),
            "w_v": np.ascontiguousarray(w_qkv[:, vcols]).astype(ml_dtypes.bfloat16),
            "b_qk": np.ascontiguousarray(b_qk.reshape(8, P).T),
            "w_proj": wp.astype(ml_dtypes.bfloat16),
            "beta": beta.reshape(1, D).astype(ml_dtypes.bfloat16),
        })
    return in_maps


def kernel(x, w_qkv, b_qkv, w_proj, b_proj, trace=False, **run_kwargs):
    global LAST_RESULTS
    nc = _build()
    in_maps = make_in_maps(x, w_qkv, b_qkv, w_proj, b_proj)
    res = run_bass_kernel_spmd(
        nc, in_maps, core_ids=list(range(NCORES)), trace=trace, **run_kwargs
    )
    LAST_RESULTS = res
    out = np.empty((B, T, D), np.float32)
    for b in range(B):
        e = res.results[2 * b]["out"].astype(np.float32)
        o = res.results[2 * b + 1]["out"].astype(np.float32)
        for Jc in range(4):
            out[b, 512 * Jc : 512 * Jc + 256] = e[256 * Jc : 256 * (Jc + 1)]
            out[b, 512 * Jc + 256 : 512 * (Jc + 1)] = o[256 * Jc : 256 * (Jc + 1)]
    return out


# revision 43
# speedup vs baseline: 1.1219x; 1.0422x over previous
"""Causal self-attention Bass kernel for 8 trn2 NeuronCores.

Problem: B=4, T=2048, D=1024, H=16 causal self-attention (qkv proj + attn + out proj).

Sharding: core c = 2*b + g handles batch b (=c//2) and head-group g (=c%2, 8 heads).

Per core (J-outer schedule):
  - x arrives pre-transposed and pre-cast to bf16 from the host as xT [D, T].
  - Attention runs J-outer: for each 512-wide tq chunk J, all 4 head pairs process
    their causal j blocks. Output chunks therefore complete progressively, letting
    the per-chunk output projection and pairwise ReduceScatter overlap attention.
  - All non-attention PE work (v proj, qk proj chunks, out proj) is split into
    small "fill units" consumed one per attention j-step, so the PE never idles
    while the scalar engine (exp) works. Deadline flushes keep the per-engine
    FIFOs deadlock-free.
  - Scores in transposed layout sT[tk, tq]; softmax denominator via a ones-column
    in the AV matmul (psum row 64). Scores / exp / AV trimmed to the causal region
    on diagonal blocks.
  - Normalization: the two [1,512] denominator rows are transposed into lanes with
    DVE 32x32 stream transposes, inverted with reciprocal_approx_fast, transposed
    back, and broadcast to 64/128 partitions via K=1 outer-product matmuls into
    psum (no DRAM round trip, no gpsimd broadcast).
  - Per-chunk ReduceScatter {2b, 2b+1} in bf16; host reassembles and casts to f32.

Precision: all matmul operands bf16, f32 psum accumulation. b_v is folded into
beta = b_proj (even core only) + w_proj_shard.T @ b_v_shard since softmax rows
sum to 1.
"""

from collections import deque
from contextlib import ExitStack

import ml_dtypes
import numpy as np

import concourse.mybir as mybir
import concourse.tile as tile
from concourse import bacc
from concourse.bass_utils import run_bass_kernel_spmd

B, T, D, H = 4, 2048, 1024, 16
HD = D // H  # 64
NCORES = 8
P = 128
f32 = mybir.dt.float32
f32r = mybir.dt.float32r
bf16 = mybir.dt.bfloat16
EXP = mybir.ActivationFunctionType.Exp

_CACHE = {}
LAST_RESULTS = None
_DEBUG_SINK = None


def _dbg(nc, name, ap):
    if _DEBUG_SINK is not None and name in _DEBUG_SINK:
        nc.sync.dma_start(_DEBUG_SINK[name].ap(), ap)


def _emit(nc, tc, xT_d, wqk_d, wv_d, bqk_d, wproj_d, beta_d, out_d):
    with ExitStack() as ctx:
        # ---------------- constants ----------------
        const = ctx.enter_context(tc.tile_pool(name="const", bufs=1))
        mask_tri = const.tile([P, P], bf16, tag="mask_tri")
        nc.gpsimd.memset(mask_tri[:], 1.0)
        nc.gpsimd.affine_select(
            out=mask_tri[:], in_=mask_tri[:],
            compare_op=mybir.AluOpType.is_ge, fill=0.0,
            base=0, pattern=[[1, P]], channel_multiplier=-1,
        )
        bq_all = const.tile([P, 8], f32, tag="bq_all")
        beta_b = const.tile([P, D], bf16, tag="beta_b")
        ones8 = const.tile([P, 8], bf16, tag="ones8")
        nc.vector.memset(ones8[:], 1.0)
        # selector for the K=2 denominator-broadcast matmul:
        # row 0 -> out partitions 0-63 (head A), row 1 -> 64-127 (head B)
        # selectors for the K=33 denominator-broadcast matmuls: selA picks
        # row 0 (head A denom), selB picks row 32 (head B denom)
        selAb = const.tile([33, 64], bf16, tag="selAb")
        nc.gpsimd.memset(selAb[:], 0.0)
        nc.gpsimd.memset(selAb[0:1, :], 1.0)
        selBb = const.tile([33, 64], bf16, tag="selBb")
        nc.gpsimd.memset(selBb[:], 0.0)
        nc.gpsimd.memset(selBb[32:33, :], 1.0)
        selA = const.tile([33, 64], f32r, tag="selA")
        nc.vector.tensor_copy(selA[:], selAb[:])
        selB = const.tile([33, 64], f32r, tag="selB")
        nc.vector.tensor_copy(selB[:], selBb[:])
        # persistent denominator scratch: rows 1-31 and 33-63 stay zero forever
        # so the K=33 broadcast matmuls see clean zeros off the two data rows
        dAB = const.tile([64, 512], f32, tag="dAB")
        nc.vector.memset(dAB[:], 0.0)
        dT = const.tile([64, 512], f32, tag="dT")
        dABr = const.tile([33, 512], f32r, tag="dABr")
        # prewarm the exp table set so the ~2.7us ACT_TABLE_LOAD overlaps the
        # x DMA instead of the first score block
        warm = const.tile([1, 8], bf16, tag="warm")
        nc.scalar.activation(warm[:], ones8[0:1, :], EXP, bias=0.0, scale=0.0)

        wpp = ctx.enter_context(tc.tile_pool(name="wpp", bufs=1))
        wproj_t = [wpp.tile([P, D], bf16, tag=f"wp{hp}", name=f"wp{hp}") for hp in range(4)]

        vv_pool = ctx.enter_context(tc.tile_pool(name="vv", bufs=1))
        vv = [vv_pool.tile([P, 520], bf16, tag=f"vv{i}", name=f"vv{i}") for i in range(16)]
        on_pool = ctx.enter_context(tc.tile_pool(name="outn", bufs=1))
        outN = [[on_pool.tile([P, 512], bf16, tag=f"outN{mp}J{J}", name=f"outN{mp}J{J}")
                 for J in range(4)] for mp in range(4)]
        ones_src = ones8[:].rearrange("p (mp h one) -> p mp h one", mp=4, h=2)
        for i in range(16):
            dst = vv[i][:].rearrange("p (mp h d) -> p mp h d", mp=4, h=2)
            nc.vector.tensor_copy(dst[:, :, :, 64:65], ones_src[:, :, :, :])

        dram = ctx.enter_context(tc.tile_pool(name="dram", bufs=1, space="DRAM"))
        rs_in = [dram.tile([512, D], bf16, tag=f"rsin{Jc}", name=f"rsin{Jc}")
                 for Jc in range(4)]
        rs_out = [dram.tile([256, D], bf16, tag=f"rsout{Jc}", name=f"rsout{Jc}")
                  for Jc in range(4)]

        qkt_pool = ctx.enter_context(tc.tile_pool(name="qkt", bufs=1))
        qkT = [qkt_pool.tile([P, T], bf16, tag=f"qkT{m}", name=f"qkT{m}") for m in range(8)]
        xt_pool = ctx.enter_context(tc.tile_pool(name="xt", bufs=1))
        xT = [xt_pool.tile([P, T], bf16, tag=f"xT{k}", name=f"xT{k}") for k in range(8)]
        wvp = ctx.enter_context(tc.tile_pool(name="wv", bufs=1))
        wv_t = [wvp.tile([P, 512], bf16, tag=f"wvt{k}", name=f"wvt{k}") for k in range(8)]
        wqkp = ctx.enter_context(tc.tile_pool(name="wqk", bufs=1))
        wq_t = [wqkp.tile([P, 1024], bf16, tag=f"wqkt{k}", name=f"wqkt{k}")
                for k in range(8)]
        wps = ctx.enter_context(tc.tile_pool(name="wps", bufs=2, space="PSUM"))

        # ---------------- critical loads first ----------------
        # w_qk columns are host-permuted to m-order (0,4,1,5,2,6,3,7) so the
        # lead-in (m=0,4) needs only the first 256 columns of each k tile.
        # Non-critical loads are emitted after the lead-in chains below.
        for k in range(8):
            nc.sync.dma_start(xT[k][:, 0:1024], xT_d.ap()[k * P : (k + 1) * P, 0:1024])
        for k in range(8):
            nc.scalar.dma_start(wv_t[k][:], wv_d.ap()[k * P : (k + 1) * P, :])
        for k in range(8):
            nc.scalar.dma_start(
                wq_t[k][:, 0:256], wqk_d.ap()[k * P : (k + 1) * P, 0:256]
            )
        nc.scalar.dma_start(bq_all[:], bqk_d.ap())
        MCOL = {0: 0, 4: 1, 1: 2, 5: 3, 2: 4, 6: 5, 3: 6, 7: 7}

        def _load_rest():
            for k in range(8):
                nc.sync.dma_start(
                    xT[k][:, 1024:2048], xT_d.ap()[k * P : (k + 1) * P, 1024:2048]
                )
            for k in range(8):
                nc.scalar.dma_start(
                    wq_t[k][:, 256:1024], wqk_d.ap()[k * P : (k + 1) * P, 256:1024]
                )
            for hp in range(4):
                nc.scalar.dma_start(
                    wproj_t[hp][:], wproj_d.ap()[hp * P : (hp + 1) * P, :]
                )
            nc.scalar.dma_start(beta_b[0:1, :], beta_d.ap())
            nc.gpsimd.partition_broadcast(beta_b[:], beta_b[0:1, :], channels=P)

        # ---------------- work units ----------------
        def v_chain(i, half):
            """half 0/1: 4 of the 8 k-matmuls for v t-tile i; evict on half 1."""
            if half == 0:
                _vbox[i] = wps.tile([P, 512], f32, tag="wp_ps", name=f"vps{i}")
            ps = _vbox[i]
            for k in range(4 * half, 4 * half + 4):
                nc.tensor.matmul(
                    ps[:], xT[k][:, i * P : (i + 1) * P], wv_t[k][:],
                    start=(k == 0), stop=(k == 7),
                )
            if half == 1:
                src = ps[:].rearrange("p (mp h d) -> p mp h d", mp=4, h=2)
                dst = vv[i][:].rearrange("p (mp h d) -> p mp h d", mp=4, h=2)
                nc.vector.tensor_copy(dst[:, :, :, 0:64], src[:, :, :, :])
        _vbox = {}

        _qkbox = {}
        def qk_chain(m, n, quarter):
            """quarter 0..3: 2 of the 8 k-matmuls for qkT[m] chunk n; evict last."""
            if quarter == 0:
                _qkbox[(m, n)] = wps.tile([P, 512], f32, tag="wp_ps", name=f"qps{m}_{n}")
            ps = _qkbox[(m, n)]
            mc = MCOL[m]
            for k in (2 * quarter, 2 * quarter + 1):
                nc.tensor.matmul(
                    ps[:], wq_t[k][:, mc * P : (mc + 1) * P],
                    xT[k][:, n * 512 : (n + 1) * 512],
                    start=(k == 0), stop=(k == 7),
                )
            if quarter == 3:
                nc.vector.tensor_scalar_add(
                    qkT[m][:, n * 512 : (n + 1) * 512], ps[:], bq_all[:, m : m + 1]
                )

        atp = ctx.enter_context(tc.tile_pool(name="atp", bufs=3))

        _pbox = {}
        def proj_unit(Jc, il, n, half):
            """half 0/1: 2 of the 4 hp-matmuls; evict + rs_in write on half 1."""
            if half == 0:
                _pbox[(Jc, il, n)] = wps.tile(
                    [P, 512], f32, tag="wp_ps", name=f"pps{Jc}_{il}_{n}"
                )
            ps = _pbox[(Jc, il, n)]
            for hp in (2 * half, 2 * half + 1):
                nc.tensor.matmul(
                    ps[:],
                    outN[hp][Jc][:, il * P : (il + 1) * P],
                    wproj_t[hp][:, n * 512 : (n + 1) * 512],
                    start=(hp == 0), stop=(hp == 3),
                )
            if half == 1:
                fin = atp.tile([P, 1024], bf16, tag="at", name="fin")
                nc.vector.tensor_add(fin[:, 0:512], ps[:], beta_b[:, n * 512 : (n + 1) * 512])
                nc.sync.dma_start(
                    rs_in[Jc][il * P : (il + 1) * P, n * 512 : (n + 1) * 512],
                    fin[:, 0:512],
                )

        def rs_unit(Jc, sub=None):
            """sub=None: whole 512-row chunk. sub=0/1: 256-row half of the
            chunk (pipelines the serial tail on chunk 3)."""
            if sub is None:
                in_ap = rs_in[Jc].opt()
                orows = slice(0, 256)
            else:
                in_ap = rs_in[Jc][sub * 256 : (sub + 1) * 256, :]
                orows = slice(sub * 128, (sub + 1) * 128)
            if globals().get("_NO_COLLECTIVE"):
                nc.sync.dma_start(
                    rs_out[Jc][orows, :],
                    rs_in[Jc][orows.start * 2 : orows.start * 2 + (orows.stop - orows.start), :],
                )
            else:
                nc.gpsimd.collective_compute(
                    "ReduceScatter", mybir.AluOpType.add,
                    replica_groups=[[0, 1], [2, 3], [4, 5], [6, 7]],
                    ins=[in_ap],
                    outs=[rs_out[Jc][orows, :]],
                )
            # gpsimd queue carries only collectives, so blocking on the RS
            # completion here cannot stall compute
            nc.gpsimd.dma_start(
                out_d.ap()[Jc * 256 + orows.start : Jc * 256 + orows.stop, :],
                rs_out[Jc][orows, :],
            )

        # fill queue: (tag, thunk) consumed one per attention j-step
        pending = deque()

        def pump():
            if pending:
                tag, thunk = pending.popleft()
                thunk()

        def flush(pred):
            """Emit from the front until no pending unit matches pred."""
            while any(pred(tag) for tag, _ in pending):
                tag, thunk = pending.popleft()
                thunk()

        def flush_all():
            while pending:
                tag, thunk = pending.popleft()
                thunk()

        # ---------------- lead-in: v tiles 0-3 and qk chunk-0 chains ----------
        for i in range(4):
            v_chain(i, 0); v_chain(i, 1)
        for m in (0, 4):
            for qq in range(4):
                qk_chain(m, 0, qq)
        _load_rest()
        for m in (1, 5, 2, 6):
            for qq in range(4):
                qk_chain(m, 0, qq)
        for m in (3, 7):
            for qq in range(4):
                pending.append((("qk", m, 0), (lambda m=m, qq=qq: qk_chain(m, 0, qq))))
        for i in range(4, 8):
            for half in range(2):
                pending.append((("v", i), (lambda i=i, h=half: v_chain(i, h))))
        for m in (0, 4, 1, 5, 2, 6, 3, 7):
            for qq in range(4):
                pending.append((("qk", m, 1), (lambda m=m, qq=qq: qk_chain(m, 1, qq))))

        # ---------------- attention: J-outer over tq chunks ----------------
        recip = ctx.enter_context(tc.tile_pool(name="recip", bufs=1))
        tmpb = ctx.enter_context(tc.tile_pool(name="tmpb", bufs=2))
        stps = ctx.enter_context(tc.tile_pool(name="stps", bufs=2, space="PSUM"))
        oups = ctx.enter_context(tc.tile_pool(name="oups", bufs=1, space="PSUM"))

        for J in range(4):
            nj = 4 * J + 4
            # correctness: everything this J's emission depends on must be
            # emitted first (per-engine FIFOs would deadlock otherwise)
            flush(lambda tag: tag[0] == "v" and tag[1] <= 4 * J + 3)
            for mp in range(4):
                flush(lambda tag: tag[0] == "qk" and tag[2] == J
                      and tag[1] in (mp, 4 + mp))
                qs, ks = qkT[mp], qkT[4 + mp]
                ouA = oups.tile([65, 512], f32, tag="ouA")
                ouB = oups.tile([65, 512], f32, tag="ouB")
                for j in range(nj):
                    sT = stps.tile([P, 1024], f32, tag="sT")
                    js = slice(j * P, (j + 1) * P)
                    i = j - 4 * J
                    c0 = 128 * i if i > 0 else 0
                    qcols = slice(J * 512 + c0, (J + 1) * 512)
                    nc.tensor.matmul(
                        sT[:, c0:512],
                        ks[0:64, js], qs[0:64, qcols],
                        start=True, stop=True, tile_position=(0, 0),
                    )
                    nc.tensor.matmul(
                        sT[:, 512 + c0 : 1024],
                        ks[64:128, js], qs[64:128, qcols],
                        start=True, stop=True, tile_position=(64, 0),
                    )
                    at = atp.tile([P, 1024], bf16, tag="at")
                    if i > 0:
                        src_v = sT[:].rearrange("p (h c) -> p h c", h=2)
                        dst_v = at[:].rearrange("p (h c) -> p h c", h=2)
                        nc.scalar.activation(
                            dst_v[:, :, c0:512], src_v[:, :, c0:512],
                            EXP, bias=0.0, scale=0.125,
                        )
                    else:
                        nc.scalar.activation(at[:], sT[:], EXP, bias=0.0, scale=0.125)
                    if i >= 0:
                        for h0 in (0, 512):
                            nc.vector.tensor_mul(
                                at[:, h0 + c0 : h0 + c0 + 128],
                                at[:, h0 + c0 : h0 + c0 + 128], mask_tri[:],
                            )
                    if mp == 0 and J == 0 and j == 0:
                        _dbg(nc, "at000", at[:])
                    nc.tensor.matmul(
                        ouA[:, c0:512], vv[j][:, 130 * mp : 130 * mp + 65],
                        at[:, c0:512],
                        start=(j == 0), stop=(j == nj - 1),
                        skip_group_check=True,
                    )
                    nc.tensor.matmul(
                        ouB[:, c0:512], vv[j][:, 130 * mp + 65 : 130 * mp + 130],
                        at[:, 512 + c0 : 1024],
                        start=(j == 0), stop=(j == nj - 1),
                        skip_group_check=True,
                    )
                    pump()
                # ---- normalize (mp, J): raw evict, lane-transposed
                # reciprocal, matmul broadcast, scale ----
                # inline: free the psum accumulators fast, split across DVE
                # (denominator rows) and ACT (value blocks) for low latency
                tb = tmpb.tile([64, 512], bf16, tag="tb")
                nc.vector.tensor_copy(dAB[0:1, :], ouA[64:65, :])
                nc.vector.tensor_copy(dAB[32:33, :], ouB[64:65, :])
                nc.scalar.copy(outN[mp][J][0:64, :], ouA[0:64, :])
                nc.scalar.copy(tb[:], ouB[0:64, :])

                def norm_tail(mp=mp, J=J, tb=tb):
                    # lane-transpose the two denominator rows (-> col 0 of
                    # each 32x32 block), invert on 64 lanes, transpose back
                    nc.vector.transpose(dT[:], dAB[:])
                    sel = dT[:].rearrange("p (b c) -> p b c", b=16)[:, :, 0:1]
                    nc.vector.reciprocal_approx_fast(sel, sel)
                    nc.vector.transpose(dAB[:], dT[:])
                    nc.vector.tensor_copy(dABr[:], dAB[0:33, :])
                    bcpA = wps.tile([P, 512], f32, tag="wp_ps", name="bcpA")
                    nc.tensor.matmul(
                        bcpA[0:64, :], selA[:], dABr[:], start=True, stop=True,
                    )
                    bcpB = wps.tile([P, 512], f32, tag="wp_ps", name="bcpB")
                    nc.tensor.matmul(
                        bcpB[0:64, :], selB[:], dABr[:], start=True, stop=True,
                    )
                    nc.vector.tensor_mul(outN[mp][J][0:64, :], outN[mp][J][0:64, :], bcpA[0:64, :])
                    nc.vector.tensor_mul(tb[:], tb[:], bcpB[0:64, :])
                    # DMA-shift head B rows up; last step, so the DVE chain
                    # never waits on the sync queue
                    nc.sync.dma_start(outN[mp][J][64:128, :], tb[:])
                # deferred off the mp-boundary critical path: runs a j-step
                # later, so the next head pair's scores aren't queued behind it
                pending.appendleft((("norm", mp, J), norm_tail))
            # ---- chunk J complete: queue its projection + ReduceScatter
            # FIRST (the RS chain is the serial tail), then v tiles and qk
            # chains needed by chunk J+2 ----
            for il in range(4):
                for n in range(2):
                    for half in range(2):
                        pending.append((("proj", J), (lambda Jc=J, il=il, n=n, h=half: proj_unit(Jc, il, n, h))))
            pending.append((("rs", J), (lambda Jc=J: rs_unit(Jc))))
            if J == 0:
                for i in range(8, 12):
                    for half in range(2):
                        pending.append((("v", i), (lambda i=i, h=half: v_chain(i, h))))
            if J == 1:
                for i in range(12, 16):
                    for half in range(2):
                        pending.append((("v", i), (lambda i=i, h=half: v_chain(i, h))))
            if J < 2:
                for m in (0, 4, 1, 5, 2, 6, 3, 7):
                    for qq in range(4):
                        pending.append((("qk", m, J + 2), (lambda m=m, qq=qq, n=J + 2: qk_chain(m, n, qq))))

        flush_all()
        _dbg(nc, "qkT0", qkT[0][:])
        _dbg(nc, "qkT4", qkT[4][:])
        _dbg(nc, "xT0", xT[0][:])
        _dbg(nc, "vv0", vv[0][:])
        if _DEBUG_SINK is not None and "outN0" in _DEBUG_SINK:
            for J in range(4):
                nc.sync.dma_start(
                    _DEBUG_SINK["outN0"].ap()[:, J * 512 : (J + 1) * 512],
                    outN[0][J][:],
                )
        for Jc in range(4):
            _dbg(nc, f"rs_in{Jc}", rs_in[Jc][:])


def _build():
    if "nc" in _CACHE:
        return _CACHE["nc"]
    nc = bacc.Bacc("TRN2", target_bir_lowering=False, debug=False, num_devices=NCORES)
    xT_d = nc.dram_tensor("xT", [D, T], bf16, kind="ExternalInput")
    wqk_d = nc.dram_tensor("w_qk", [D, 1024], bf16, kind="ExternalInput")
    wv_d = nc.dram_tensor("w_v", [D, 512], bf16, kind="ExternalInput")
    bqk_d = nc.dram_tensor("b_qk", [P, 8], f32, kind="ExternalInput")
    wproj_d = nc.dram_tensor("w_proj", [512, D], bf16, kind="ExternalInput")
    beta_d = nc.dram_tensor("beta", [1, D], bf16, kind="ExternalInput")
    out_d = nc.dram_tensor("out", [T // 2, D], bf16, kind="ExternalOutput")
    with tile.TileContext(nc) as tc:
        _emit(nc, tc, xT_d, wqk_d, wv_d, bqk_d, wproj_d, beta_d, out_d)
    nc.compile()
    _CACHE["nc"] = nc
    return nc


def make_in_maps(x, w_qkv, b_qkv, w_proj, b_proj):
    x = np.asarray(x, np.float32)
    w_qkv = np.asarray(w_qkv, np.float32)
    b_qkv = np.asarray(b_qkv, np.float32)
    w_proj = np.asarray(w_proj, np.float32)
    b_proj = np.asarray(b_proj, np.float32)
    in_maps = []
    for c in range(NCORES):
        b, g = c // 2, c % 2
        qcols = slice(g * 512, (g + 1) * 512)
        kcols = slice(D + g * 512, D + (g + 1) * 512)
        vcols = slice(2 * D + g * 512, 2 * D + (g + 1) * 512)
        w_qk = np.concatenate([w_qkv[:, qcols], w_qkv[:, kcols]], axis=1)
        # permute 128-col blocks to m-order (0,4,1,5,2,6,3,7): the kernel's
        # lead-in then only needs the first 256 columns per k tile
        blk = w_qk.reshape(D, 8, P)
        w_qk = blk[:, [0, 4, 1, 5, 2, 6, 3, 7], :].reshape(D, 1024)
        b_qk = np.concatenate([b_qkv[qcols], b_qkv[kcols]])
        wp = np.ascontiguousarray(w_proj[g * 512 : (g + 1) * 512, :])
        beta = wp.T @ b_qkv[vcols]
        if g == 0:
            beta = beta + b_proj
        in_maps.append({
            "xT": np.ascontiguousarray(x[b].T).astype(ml_dtypes.bfloat16),
            "w_qk": np.ascontiguousarray(w_qk).astype(ml_dtypes.bfloat16),
            "w_v": np.ascontiguousarray(w_qkv[:, vcols]).astype(ml_dtypes.bfloat16),
            "b_qk": np.ascontiguousarray(b_qk.reshape(8, P).T),
            "w_proj": wp.astype(ml_dtypes.bfloat16),
            "beta": beta.reshape(1, D).astype(ml_dtypes.bfloat16),
        })
    return in_maps


def kernel(x, w_qkv, b_qkv, w_proj, b_proj, trace=False, **run_kwargs):
    global LAST_RESULTS
    nc = _build()
    in_maps = make_in_maps(x, w_qkv, b_qkv, w_proj, b_proj)
    res = run_bass_kernel_spmd(
        nc, in_maps, core_ids=list(range(NCORES)), trace=trace, **run_kwargs
    )
    LAST_RESULTS = res
    out = np.empty((B, T, D), np.float32)
    for b in range(B):
        e = res.results[2 * b]["out"].astype(np.float32)
        o = res.results[2 * b + 1]["out"].astype(np.float32)
        for Jc in range(4):
            out[b, 512 * Jc : 512 * Jc + 256] = e[256 * Jc : 256 * (Jc + 1)]
            out[b, 512 * Jc + 256 : 512 * (Jc + 1)] = o[256 * Jc : 256 * (Jc + 1)]
    return out


# revision 45
# speedup vs baseline: 1.1281x; 1.0055x over previous
"""Causal self-attention Bass kernel for 8 trn2 NeuronCores.

Problem: B=4, T=2048, D=1024, H=16 causal self-attention (qkv proj + attn + out proj).

Sharding: core c = 2*b + g handles batch b (=c//2) and head-group g (=c%2, 8 heads).

Per core (J-outer schedule):
  - x arrives pre-transposed and pre-cast to bf16 from the host as xT [D, T].
  - Attention runs J-outer: for each 512-wide tq chunk J, all 4 head pairs process
    their causal j blocks. Output chunks therefore complete progressively, letting
    the per-chunk output projection and pairwise ReduceScatter overlap attention.
  - All non-attention PE work (v proj, qk proj chunks, out proj) is split into
    small "fill units" consumed one per attention j-step, so the PE never idles
    while the scalar engine (exp) works. Deadline flushes keep the per-engine
    FIFOs deadlock-free.
  - Scores in transposed layout sT[tk, tq]; softmax denominator via a ones-column
    in the AV matmul (psum row 64). Scores / exp / AV trimmed to the causal region
    on diagonal blocks.
  - Normalization: the two [1,512] denominator rows are transposed into lanes with
    DVE 32x32 stream transposes, inverted with reciprocal_approx_fast, transposed
    back, and broadcast to 64/128 partitions via K=1 outer-product matmuls into
    psum (no DRAM round trip, no gpsimd broadcast).
  - Per-chunk ReduceScatter {2b, 2b+1} in bf16; host reassembles and casts to f32.

Precision: all matmul operands bf16, f32 psum accumulation. b_v is folded into
beta = b_proj (even core only) + w_proj_shard.T @ b_v_shard since softmax rows
sum to 1.
"""

from collections import deque
from contextlib import ExitStack

import ml_dtypes
import numpy as np

import concourse.mybir as mybir
import concourse.tile as tile
from concourse import bacc
from concourse.bass_utils import run_bass_kernel_spmd

B, T, D, H = 4, 2048, 1024, 16
HD = D // H  # 64
NCORES = 8
P = 128
f32 = mybir.dt.float32
f32r = mybir.dt.float32r
bf16 = mybir.dt.bfloat16
EXP = mybir.ActivationFunctionType.Exp

_CACHE = {}
LAST_RESULTS = None
_DEBUG_SINK = None


def _dbg(nc, name, ap):
    if _DEBUG_SINK is not None and name in _DEBUG_SINK:
        nc.sync.dma_start(_DEBUG_SINK[name].ap(), ap)


def _emit(nc, tc, xT_d, wqk_d, wv_d, bqk_d, wproj_d, beta_d, out_d):
    with ExitStack() as ctx:
        # ---------------- constants ----------------
        const = ctx.enter_context(tc.tile_pool(name="const", bufs=1))
        mask_tri = const.tile([P, P], bf16, tag="mask_tri")
        nc.gpsimd.memset(mask_tri[:], 1.0)
        nc.gpsimd.affine_select(
            out=mask_tri[:], in_=mask_tri[:],
            compare_op=mybir.AluOpType.is_ge, fill=0.0,
            base=0, pattern=[[1, P]], channel_multiplier=-1,
        )
        bq_all = const.tile([P, 8], f32, tag="bq_all")
        beta_b = const.tile([P, D], bf16, tag="beta_b")
        ones8 = const.tile([P, 8], bf16, tag="ones8")
        nc.vector.memset(ones8[:], 1.0)
        # selector for the K=2 denominator-broadcast matmul:
        # row 0 -> out partitions 0-63 (head A), row 1 -> 64-127 (head B)
        # selectors for the K=33 denominator-broadcast matmuls: selA picks
        # row 0 (head A denom), selB picks row 32 (head B denom)
        selAb = const.tile([33, 64], bf16, tag="selAb")
        nc.gpsimd.memset(selAb[:], 0.0)
        nc.gpsimd.memset(selAb[0:1, :], 1.0)
        selBb = const.tile([33, 64], bf16, tag="selBb")
        nc.gpsimd.memset(selBb[:], 0.0)
        nc.gpsimd.memset(selBb[32:33, :], 1.0)
        selA = const.tile([33, 64], f32r, tag="selA")
        nc.vector.tensor_copy(selA[:], selAb[:])
        selB = const.tile([33, 64], f32r, tag="selB")
        nc.vector.tensor_copy(selB[:], selBb[:])
        # persistent denominator scratch: rows 1-31 and 33-63 stay zero forever
        # so the K=33 broadcast matmuls see clean zeros off the two data rows
        dAB = const.tile([64, 512], f32, tag="dAB")
        nc.vector.memset(dAB[:], 0.0)
        dT = const.tile([64, 512], f32, tag="dT")
        dABr = const.tile([33, 512], f32r, tag="dABr")
        # prewarm the exp table set so the ~2.7us ACT_TABLE_LOAD overlaps the
        # x DMA instead of the first score block
        warm = const.tile([1, 8], bf16, tag="warm")
        nc.scalar.activation(warm[:], ones8[0:1, :], EXP, bias=0.0, scale=0.0)

        wpp = ctx.enter_context(tc.tile_pool(name="wpp", bufs=1))
        wproj_t = [wpp.tile([P, D], bf16, tag=f"wp{hp}", name=f"wp{hp}") for hp in range(4)]

        vv_pool = ctx.enter_context(tc.tile_pool(name="vv", bufs=1))
        vv = [vv_pool.tile([P, 520], bf16, tag=f"vv{i}", name=f"vv{i}") for i in range(16)]
        on_pool = ctx.enter_context(tc.tile_pool(name="outn", bufs=1))
        outN = [[on_pool.tile([P, 512], bf16, tag=f"outN{mp}J{J}", name=f"outN{mp}J{J}")
                 for J in range(4)] for mp in range(4)]
        ones_src = ones8[:].rearrange("p (mp h one) -> p mp h one", mp=4, h=2)
        for i in range(16):
            dst = vv[i][:].rearrange("p (mp h d) -> p mp h d", mp=4, h=2)
            nc.vector.tensor_copy(dst[:, :, :, 64:65], ones_src[:, :, :, :])

        dram = ctx.enter_context(tc.tile_pool(name="dram", bufs=1, space="DRAM"))
        rs_in = [dram.tile([512, D], bf16, tag=f"rsin{Jc}", name=f"rsin{Jc}")
                 for Jc in range(4)]
        rs_out = [dram.tile([256, D], bf16, tag=f"rsout{Jc}", name=f"rsout{Jc}")
                  for Jc in range(4)]

        qkt_pool = ctx.enter_context(tc.tile_pool(name="qkt", bufs=1))
        qkT = [qkt_pool.tile([P, T], bf16, tag=f"qkT{m}", name=f"qkT{m}") for m in range(8)]
        xt_pool = ctx.enter_context(tc.tile_pool(name="xt", bufs=1))
        xT = [xt_pool.tile([P, T], bf16, tag=f"xT{k}", name=f"xT{k}") for k in range(8)]
        wvp = ctx.enter_context(tc.tile_pool(name="wv", bufs=1))
        wv_t = [wvp.tile([P, 512], bf16, tag=f"wvt{k}", name=f"wvt{k}") for k in range(8)]
        wqkp = ctx.enter_context(tc.tile_pool(name="wqk", bufs=1))
        wq_t = [wqkp.tile([P, 1024], bf16, tag=f"wqkt{k}", name=f"wqkt{k}")
                for k in range(8)]
        wps = ctx.enter_context(tc.tile_pool(name="wps", bufs=2, space="PSUM"))

        # ---------------- critical loads first ----------------
        # w_qk columns are host-permuted to m-order (0,4,1,5,2,6,3,7) so the
        # lead-in (m=0,4) needs only the first 256 columns of each k tile.
        # Non-critical loads are emitted after the lead-in chains below.
        for k in range(8):
            nc.sync.dma_start(xT[k][:, 0:1024], xT_d.ap()[k * P : (k + 1) * P, 0:1024])
        for k in range(8):
            nc.scalar.dma_start(
                wq_t[k][:, 0:256], wqk_d.ap()[k * P : (k + 1) * P, 0:256]
            )
        nc.scalar.dma_start(bq_all[:], bqk_d.ap())
        for k in range(8):
            nc.scalar.dma_start(wv_t[k][:], wv_d.ap()[k * P : (k + 1) * P, :])
        MCOL = {0: 0, 4: 1, 1: 2, 5: 3, 2: 4, 6: 5, 3: 6, 7: 7}

        def _load_rest():
            for k in range(8):
                nc.sync.dma_start(
                    xT[k][:, 1024:2048], xT_d.ap()[k * P : (k + 1) * P, 1024:2048]
                )
            for k in range(8):
                nc.scalar.dma_start(
                    wq_t[k][:, 256:1024], wqk_d.ap()[k * P : (k + 1) * P, 256:1024]
                )
            for hp in range(4):
                nc.scalar.dma_start(
                    wproj_t[hp][:], wproj_d.ap()[hp * P : (hp + 1) * P, :]
                )
            nc.scalar.dma_start(beta_b[0:1, :], beta_d.ap())
            nc.gpsimd.partition_broadcast(beta_b[:], beta_b[0:1, :], channels=P)

        # ---------------- work units ----------------
        def v_chain(i, half):
            """half 0/1: 4 of the 8 k-matmuls for v t-tile i; evict on half 1."""
            if half == 0:
                _vbox[i] = wps.tile([P, 512], f32, tag="wp_ps", name=f"vps{i}")
            ps = _vbox[i]
            for k in range(4 * half, 4 * half + 4):
                nc.tensor.matmul(
                    ps[:], xT[k][:, i * P : (i + 1) * P], wv_t[k][:],
                    start=(k == 0), stop=(k == 7),
                )
            if half == 1:
                src = ps[:].rearrange("p (mp h d) -> p mp h d", mp=4, h=2)
                dst = vv[i][:].rearrange("p (mp h d) -> p mp h d", mp=4, h=2)
                nc.vector.tensor_copy(dst[:, :, :, 0:64], src[:, :, :, :])
        _vbox = {}

        _qkbox = {}
        def qk_chain(m, n, quarter):
            """quarter 0..3: 2 of the 8 k-matmuls for qkT[m] chunk n; evict last."""
            if quarter == 0:
                _qkbox[(m, n)] = wps.tile([P, 512], f32, tag="wp_ps", name=f"qps{m}_{n}")
            ps = _qkbox[(m, n)]
            mc = MCOL[m]
            for k in (2 * quarter, 2 * quarter + 1):
                nc.tensor.matmul(
                    ps[:], wq_t[k][:, mc * P : (mc + 1) * P],
                    xT[k][:, n * 512 : (n + 1) * 512],
                    start=(k == 0), stop=(k == 7),
                )
            if quarter == 3:
                nc.vector.tensor_scalar_add(
                    qkT[m][:, n * 512 : (n + 1) * 512], ps[:], bq_all[:, m : m + 1]
                )

        atp = ctx.enter_context(tc.tile_pool(name="atp", bufs=3))

        _pbox = {}
        def proj_unit(Jc, il, n, half):
            """half 0/1: 2 of the 4 hp-matmuls; evict + rs_in write on half 1."""
            if half == 0:
                _pbox[(Jc, il, n)] = wps.tile(
                    [P, 512], f32, tag="wp_ps", name=f"pps{Jc}_{il}_{n}"
                )
            ps = _pbox[(Jc, il, n)]
            for hp in (2 * half, 2 * half + 1):
                nc.tensor.matmul(
                    ps[:],
                    outN[hp][Jc][:, il * P : (il + 1) * P],
                    wproj_t[hp][:, n * 512 : (n + 1) * 512],
                    start=(hp == 0), stop=(hp == 3),
                )
            if half == 1:
                fin = atp.tile([P, 1024], bf16, tag="at", name="fin")
                nc.vector.tensor_add(fin[:, 0:512], ps[:], beta_b[:, n * 512 : (n + 1) * 512])
                nc.sync.dma_start(
                    rs_in[Jc][il * P : (il + 1) * P, n * 512 : (n + 1) * 512],
                    fin[:, 0:512],
                )

        def rs_unit(Jc, sub=None):
            """sub=None: whole 512-row chunk. sub=0/1: 256-row half of the
            chunk (pipelines the serial tail on chunk 3)."""
            if sub is None:
                in_ap = rs_in[Jc].opt()
                orows = slice(0, 256)
            else:
                in_ap = rs_in[Jc][sub * 256 : (sub + 1) * 256, :]
                orows = slice(sub * 128, (sub + 1) * 128)
            if globals().get("_NO_COLLECTIVE"):
                nc.sync.dma_start(
                    rs_out[Jc][orows, :],
                    rs_in[Jc][orows.start * 2 : orows.start * 2 + (orows.stop - orows.start), :],
                )
            else:
                nc.gpsimd.collective_compute(
                    "ReduceScatter", mybir.AluOpType.add,
                    replica_groups=[[0, 1], [2, 3], [4, 5], [6, 7]],
                    ins=[in_ap],
                    outs=[rs_out[Jc][orows, :]],
                )
            # gpsimd queue carries only collectives, so blocking on the RS
            # completion here cannot stall compute
            nc.gpsimd.dma_start(
                out_d.ap()[Jc * 256 + orows.start : Jc * 256 + orows.stop, :],
                rs_out[Jc][orows, :],
            )

        # fill queue: (tag, thunk) consumed one per attention j-step
        pending = deque()

        def pump():
            if pending:
                tag, thunk = pending.popleft()
                thunk()

        def flush(pred):
            """Emit from the front until no pending unit matches pred."""
            while any(pred(tag) for tag, _ in pending):
                tag, thunk = pending.popleft()
                thunk()

        def flush_all():
            while pending:
                tag, thunk = pending.popleft()
                thunk()

        # PE warm-up: ~60 dependency-free matmuls keep the HAM clock gate at
        # full rate through the initial DMA window, so the lead-in chains run
        # at 2.4 GHz instead of 1.2
        warmpe = const.tile([P, 512], bf16, tag="warmpe")
        nc.vector.memset(warmpe[:], 0.0)
        for w in range(60):
            wp = wps.tile([P, 512], f32, tag="wp_ps", name=f"warm{w}")
            nc.tensor.matmul(wp[:], mask_tri[:], warmpe[:], start=True, stop=True)

        # ---------------- lead-in: first score-gating chains, then v ----------
        for m in (0, 4):
            for qq in range(4):
                qk_chain(m, 0, qq)
        for i in range(4):
            v_chain(i, 0); v_chain(i, 1)
        _load_rest()
        for m in (1, 5, 2, 6):
            for qq in range(4):
                qk_chain(m, 0, qq)
        for m in (3, 7):
            for qq in range(4):
                pending.append((("qk", m, 0), (lambda m=m, qq=qq: qk_chain(m, 0, qq))))
        for i in range(4, 8):
            for half in range(2):
                pending.append((("v", i), (lambda i=i, h=half: v_chain(i, h))))
        for m in (0, 4, 1, 5, 2, 6, 3, 7):
            for qq in range(4):
                pending.append((("qk", m, 1), (lambda m=m, qq=qq: qk_chain(m, 1, qq))))

        # ---------------- attention: J-outer over tq chunks ----------------
        recip = ctx.enter_context(tc.tile_pool(name="recip", bufs=1))
        tmpb = ctx.enter_context(tc.tile_pool(name="tmpb", bufs=2))
        stps = ctx.enter_context(tc.tile_pool(name="stps", bufs=2, space="PSUM"))
        oups = ctx.enter_context(tc.tile_pool(name="oups", bufs=1, space="PSUM"))

        for J in range(4):
            nj = 4 * J + 4
            # correctness: everything this J's emission depends on must be
            # emitted first (per-engine FIFOs would deadlock otherwise)
            flush(lambda tag: tag[0] == "v" and tag[1] <= 4 * J + 3)
            for mp in range(4):
                flush(lambda tag: tag[0] == "qk" and tag[2] == J
                      and tag[1] in (mp, 4 + mp))
                qs, ks = qkT[mp], qkT[4 + mp]
                ouA = oups.tile([65, 512], f32, tag="ouA")
                ouB = oups.tile([65, 512], f32, tag="ouB")
                for j in range(nj):
                    sT = stps.tile([P, 1024], f32, tag="sT")
                    js = slice(j * P, (j + 1) * P)
                    i = j - 4 * J
                    c0 = 128 * i if i > 0 else 0
                    qcols = slice(J * 512 + c0, (J + 1) * 512)
                    nc.tensor.matmul(
                        sT[:, c0:512],
                        ks[0:64, js], qs[0:64, qcols],
                        start=True, stop=True, tile_position=(0, 0),
                    )
                    nc.tensor.matmul(
                        sT[:, 512 + c0 : 1024],
                        ks[64:128, js], qs[64:128, qcols],
                        start=True, stop=True, tile_position=(64, 0),
                    )
                    at = atp.tile([P, 1024], bf16, tag="at")
                    if i > 0:
                        src_v = sT[:].rearrange("p (h c) -> p h c", h=2)
                        dst_v = at[:].rearrange("p (h c) -> p h c", h=2)
                        nc.scalar.activation(
                            dst_v[:, :, c0:512], src_v[:, :, c0:512],
                            EXP, bias=0.0, scale=0.125,
                        )
                    else:
                        nc.scalar.activation(at[:], sT[:], EXP, bias=0.0, scale=0.125)
                    if i >= 0:
                        for h0 in (0, 512):
                            nc.vector.tensor_mul(
                                at[:, h0 + c0 : h0 + c0 + 128],
                                at[:, h0 + c0 : h0 + c0 + 128], mask_tri[:],
                            )
                    if mp == 0 and J == 0 and j == 0:
                        _dbg(nc, "at000", at[:])
                    nc.tensor.matmul(
                        ouA[:, c0:512], vv[j][:, 130 * mp : 130 * mp + 65],
                        at[:, c0:512],
                        start=(j == 0), stop=(j == nj - 1),
                        skip_group_check=True,
                    )
                    nc.tensor.matmul(
                        ouB[:, c0:512], vv[j][:, 130 * mp + 65 : 130 * mp + 130],
                        at[:, 512 + c0 : 1024],
                        start=(j == 0), stop=(j == nj - 1),
                        skip_group_check=True,
                    )
                    pump()
                # ---- normalize (mp, J): raw evict, lane-transposed
                # reciprocal, matmul broadcast, scale ----
                # inline: free the psum accumulators fast, split across DVE
                # (denominator rows) and ACT (value blocks) for low latency
                tb = tmpb.tile([64, 512], bf16, tag="tb")
                nc.vector.tensor_copy(dAB[0:1, :], ouA[64:65, :])
                nc.vector.tensor_copy(dAB[32:33, :], ouB[64:65, :])
                nc.scalar.copy(outN[mp][J][0:64, :], ouA[0:64, :])
                nc.scalar.copy(tb[:], ouB[0:64, :])

                def norm_tail(mp=mp, J=J, tb=tb):
                    # lane-transpose the two denominator rows (-> col 0 of
                    # each 32x32 block), invert on 64 lanes, transpose back
                    nc.vector.transpose(dT[:], dAB[:])
                    sel = dT[:].rearrange("p (b c) -> p b c", b=16)[:, :, 0:1]
                    nc.vector.reciprocal_approx_fast(sel, sel)
                    nc.vector.transpose(dAB[:], dT[:])
                    nc.vector.tensor_copy(dABr[:], dAB[0:33, :])
                    bcpA = wps.tile([P, 512], f32, tag="wp_ps", name="bcpA")
                    nc.tensor.matmul(
                        bcpA[0:64, :], selA[:], dABr[:], start=True, stop=True,
                    )
                    bcpB = wps.tile([P, 512], f32, tag="wp_ps", name="bcpB")
                    nc.tensor.matmul(
                        bcpB[0:64, :], selB[:], dABr[:], start=True, stop=True,
                    )
                    nc.vector.tensor_mul(outN[mp][J][0:64, :], outN[mp][J][0:64, :], bcpA[0:64, :])
                    nc.vector.tensor_mul(tb[:], tb[:], bcpB[0:64, :])
                    # DMA-shift head B rows up; last step, so the DVE chain
                    # never waits on the sync queue
                    nc.sync.dma_start(outN[mp][J][64:128, :], tb[:])
                # deferred off the mp-boundary critical path: runs a j-step
                # later, so the next head pair's scores aren't queued behind it
                pending.appendleft((("norm", mp, J), norm_tail))
            # ---- chunk J complete: queue its projection + ReduceScatter
            # FIRST (the RS chain is the serial tail), then v tiles and qk
            # chains needed by chunk J+2 ----
            for il in range(4):
                for n in range(2):
                    for half in range(2):
                        pending.append((("proj", J), (lambda Jc=J, il=il, n=n, h=half: proj_unit(Jc, il, n, h))))
            pending.append((("rs", J), (lambda Jc=J: rs_unit(Jc))))
            if J == 0:
                for i in range(8, 12):
                    for half in range(2):
                        pending.append((("v", i), (lambda i=i, h=half: v_chain(i, h))))
            if J == 1:
                for i in range(12, 16):
                    for half in range(2):
                        pending.append((("v", i), (lambda i=i, h=half: v_chain(i, h))))
            if J < 2:
                for m in (0, 4, 1, 5, 2, 6, 3, 7):
                    for qq in range(4):
                        pending.append((("qk", m, J + 2), (lambda m=m, qq=qq, n=J + 2: qk_chain(m, n, qq))))

        flush_all()
        _dbg(nc, "qkT0", qkT[0][:])
        _dbg(nc, "qkT4", qkT[4][:])
        _dbg(nc, "xT0", xT[0][:])
        _dbg(nc, "vv0", vv[0][:])
        if _DEBUG_SINK is not None and "outN0" in _DEBUG_SINK:
            for J in range(4):
                nc.sync.dma_start(
                    _DEBUG_SINK["outN0"].ap()[:, J * 512 : (J + 1) * 512],
                    outN[0][J][:],
                )
        for Jc in range(4):
            _dbg(nc, f"rs_in{Jc}", rs_in[Jc][:])


def _build():
    if "nc" in _CACHE:
        return _CACHE["nc"]
    nc = bacc.Bacc("TRN2", target_bir_lowering=False, debug=False, num_devices=NCORES)
    xT_d = nc.dram_tensor("xT", [D, T], bf16, kind="ExternalInput")
    wqk_d = nc.dram_tensor("w_qk", [D, 1024], bf16, kind="ExternalInput")
    wv_d = nc.dram_tensor("w_v", [D, 512], bf16, kind="ExternalInput")
    bqk_d = nc.dram_tensor("b_qk", [P, 8], f32, kind="ExternalInput")
    wproj_d = nc.dram_tensor("w_proj", [512, D], bf16, kind="ExternalInput")
    beta_d = nc.dram_tensor("beta", [1, D], bf16, kind="ExternalInput")
    out_d = nc.dram_tensor("out", [T // 2, D], bf16, kind="ExternalOutput")
    with tile.TileContext(nc) as tc:
        _emit(nc, tc, xT_d, wqk_d, wv_d, bqk_d, wproj_d, beta_d, out_d)
    nc.compile()
    _CACHE["nc"] = nc
    return nc


def make_in_maps(x, w_qkv, b_qkv, w_proj, b_proj):
    x = np.asarray(x, np.float32)
    w_qkv = np.asarray(w_qkv, np.float32)
    b_qkv = np.asarray(b_qkv, np.float32)
    w_proj = np.asarray(w_proj, np.float32)
    b_proj = np.asarray(b_proj, np.float32)
    in_maps = []
    for c in range(NCORES):
        b, g = c // 2, c % 2
        qcols = slice(g * 512, (g + 1) * 512)
        kcols = slice(D + g * 512, D + (g + 1) * 512)
        vcols = slice(2 * D + g * 512, 2 * D + (g + 1) * 512)
        w_qk = np.concatenate([w_qkv[:, qcols], w_qkv[:, kcols]], axis=1)
        # permute 128-col blocks to m-order (0,4,1,5,2,6,3,7): the kernel's
        # lead-in then only needs the first 256 columns per k tile
        blk = w_qk.reshape(D, 8, P)
        w_qk = blk[:, [0, 4, 1, 5, 2, 6, 3, 7], :].reshape(D, 1024)
        b_qk = np.concatenate([b_qkv[qcols], b_qkv[kcols]])
        wp = np.ascontiguousarray(w_proj[g * 512 : (g + 1) * 512, :])
        beta = wp.T @ b_qkv[vcols]
        if g == 0:
            beta = beta + b_proj
        in_maps.append({
            "xT": np.ascontiguousarray(x[b].T).astype(ml_dtypes.bfloat16),
            "w_qk": np.ascontiguousarray(w_qk).astype(ml_dtypes.bfloat16),
            "w_v": np.ascontiguousarray(w_qkv[:, vcols]).astype(ml_dtypes.bfloat16),
            "b_qk": np.ascontiguousarray(b_qk.reshape(8, P).T),
            "w_proj": wp.astype(ml_dtypes.bfloat16),
            "beta": beta.reshape(1, D).astype(ml_dtypes.bfloat16),
        })
    return in_maps


def kernel(x, w_qkv, b_qkv, w_proj, b_proj, trace=False, **run_kwargs):
    global LAST_RESULTS
    nc = _build()
    in_maps = make_in_maps(x, w_qkv, b_qkv, w_proj, b_proj)
    res = run_bass_kernel_spmd(
        nc, in_maps, core_ids=list(range(NCORES)), trace=trace, **run_kwargs
    )
    LAST_RESULTS = res
    out = np.empty((B, T, D), np.float32)
    for b in range(B):
        e = res.results[2 * b]["out"].astype(np.float32)
        o = res.results[2 * b + 1]["out"].astype(np.float32)
        for Jc in range(4):
            out[b, 512 * Jc : 512 * Jc + 256] = e[256 * Jc : 256 * (Jc + 1)]
            out[b, 512 * Jc + 256 : 512 * (Jc + 1)] = o[256 * Jc : 256 * (Jc + 1)]
    return out
